# revision 7
# baseline (speedup 1.0000x reference)
"""Sliding-window attention (B=2,S=2048,H=8,D=64,W=128) on 8 trn2 cores.

Sharding: 16 (b,h) pairs -> 8 cores x 2 adjacent heads (same b). Per core the
inputs are [S, 2*64] fp32 slabs (adjacent heads => 512B contiguous DRAM runs).

Per-core kernel (all on-chip work in fp16, accumulation fp32):
  prep:   cast-DMA slabs to fp16; PE-transpose Q,K pairs -> QT/KT [128,2048]
          (rows 0:64 head0^T, 64:128 head1^T); build V' [128,16,65] per head
          with a ones column (row-sum trick for the softmax denominator).
  scores: per key-block kb: scoresT strip [128 k, <=384 q] = KT_blk.T @ QT_slab
          (one matmul, K=64 contraction); exp via ACT (scale=1/8 folded in),
          band masks as 0/1 triangular multiplies on the two edge sections.
  pv:     per query-block t: out[128,65] = sum_kb P^T_sec.T @ V'[kb] in PSUM;
          col 64 = softmax denom; normalize with DVE reciprocal + scalar mul.
"""

import numpy as np

B, S, H, D = 2, 2048, 8, 64
W = 128
NB = S // 128  # 16 seq blocks

_cached = {}


def _install_drain_split():
    """Walrus in this container encodes ~1 sync-wait per CTRL instruction; the
    Tile end-of-kernel drain aggregates one wait per live semaphore and fails
    codegen. Split the waits across single-wait NoOps on the sync engine."""
    import concourse.tile as tile
    from bass_rust import VectorClock, ScopedClock

    def _split_drain_and_barrier(self, tick_clock, wait_clock):
        gc = tick_clock.global_clock
        vals = [gc.peek_next(i) - 1 for i in range(27)]
        for i, v in [(i, v) for i, v in enumerate(vals) if v > 0]:
            sub = VectorClock()
            sub.require_at_least(i, v)
            nop_inst = self.nc.sync.nop(nofuse=True)
            wait_clock.add_sem_waits(nop_inst.ins, ScopedClock({None: sub}))
        self.nc.sync.drain()
        self.nc.all_engine_barrier()
        assert self.sems is not None
        popped = self.nc._tile_sem_poison_stack.pop()
        assert popped is self._sem_poison
        self.nc.clear_and_free_semaphores(list(self.sems.allocated().values()))
        self.nc.all_engine_barrier()

    tile.TileContext._drain_and_barrier = _split_drain_and_barrier


def _build():
    import concourse.bass as bass
    import concourse.mybir as mybir
    import concourse.tile as tile

    _install_drain_split()

    fp16 = mybir.dt.float16
    fp32 = mybir.dt.float32

    nc = bass.Bass()
    q_in = nc.dram_tensor("q", [S, 128], fp32, kind="ExternalInput")
    k_in = nc.dram_tensor("k", [S, 128], fp32, kind="ExternalInput")
    v_in = nc.dram_tensor("v", [S, 128], fp32, kind="ExternalInput")
    ident_in = nc.dram_tensor("ident", [128, 128], fp32, kind="ExternalInput")
    mband_in = nc.dram_tensor("mband", [128, 384], fp16, kind="ExternalInput")
    out = nc.dram_tensor("out", [S, 128], fp32, kind="ExternalOutput")

    with tile.TileContext(nc) as tc:
        consts = tc.tile_pool(name="consts", bufs=1).__enter__()
        stage = tc.tile_pool(name="stage", bufs=4).__enter__()
        tp_psum = tc.tile_pool(name="tp_psum", bufs=2, space="PSUM").__enter__()
        sc_psum = tc.tile_pool(name="sc_psum", bufs=2, space="PSUM").__enter__()
        pv_psum = tc.tile_pool(name="pv_psum", bufs=4, space="PSUM").__enter__()
        strips = tc.tile_pool(name="strips", bufs=16).__enter__()
        small = tc.tile_pool(name="small", bufs=4).__enter__()
        outs = tc.tile_pool(name="outs", bufs=4).__enter__()

        identf = consts.tile([128, 128], fp32, tag="ident")
        m_band = consts.tile([128, 384], fp16, tag="mband")
        nc.sync.dma_start(out=identf, in_=ident_in[:])
        nc.sync.dma_start(out=m_band, in_=mband_in[:])

        qt = consts.tile([128, S], fp16, tag="qt")
        kt = consts.tile([128, S], fp16, tag="kt")
        q_all = consts.tile([128, NB, 128], fp32, tag="q_all")
        k_all = consts.tile([128, NB, 128], fp32, tag="k_all")
        v_all = consts.tile([128, NB, 128], fp32, tag="v_all")
        vp = [consts.tile([128, NB, 65], fp16, tag=f"vp{bh}") for bh in range(2)]

        # ---- prep: chunked cast loads pipelined with pair transposes ----
        q_view = q_in.rearrange("(n p) c -> p n c", p=128)
        k_view = k_in.rearrange("(n p) c -> p n c", p=128)
        v_view = v_in.rearrange("(n p) c -> p n c", p=128)
        nc.sync.dma_start(out=q_all, in_=q_view)
        nc.sync.dma_start(out=k_all, in_=k_view)
        nc.sync.dma_start(out=v_all, in_=v_view)
        for i in range(NB):
            sl = slice(i * 128, (i + 1) * 128)
            qtp = tp_psum.tile([128, 128], fp32, tag="tp")
            nc.tensor.transpose(qtp, q_all[:, i, :], identf)
            nc.scalar.copy(out=qt[:, sl], in_=qtp)
            ktp = tp_psum.tile([128, 128], fp32, tag="tp")
            nc.tensor.transpose(ktp, k_all[:, i, :], identf)
            nc.vector.tensor_copy(out=kt[:, sl], in_=ktp)
        for bh in range(2):
            nc.vector.tensor_copy(
                out=vp[bh][:, :, 0:64], in_=v_all[:, :, bh * 64 : bh * 64 + 64]
            )
            nc.vector.memset(vp[bh][:, :, 64:65], 1.0)

        # ---- scores: one strip per (kb, bh) ----
        # strip(kb) covers query blocks t in [t0, t1]; section offsets 128*(t-t0)
        strip_tiles = {}
        for kb in range(NB):
            t0, t1 = max(0, kb - 1), min(NB - 1, kb + 1)
            w = (t1 - t0 + 1) * 128
            qsl = slice(t0 * 128, (t1 + 1) * 128)
            ksl = slice(kb * 128, (kb + 1) * 128)
            for bh in range(2):
                rows = slice(bh * 64, bh * 64 + 64)
                sc = sc_psum.tile([128, 384], fp32, tag="sc")
                nc.tensor.matmul(
                    sc[:, :w], kt[rows, ksl], qt[rows, qsl], start=True, stop=True
                )
                st = strips.tile([128, 384], fp16, tag="strip")
                nc.scalar.activation(
                    out=st[:, :w], in_=sc[:, :w],
                    func=mybir.ActivationFunctionType.Exp, scale=0.125,
                )
                if kb > 0:  # section t=kb-1: keep k <= q
                    nc.vector.tensor_mul(st[:, 0:128], st[:, 0:128], m_band[:, 0:128])
                if kb < NB - 1:  # section t=kb+1: keep k >= q
                    off = (kb + 1 - t0) * 128
                    nc.vector.tensor_mul(
                        st[:, off : off + 128], st[:, off : off + 128],
                        m_band[:, 256:384],
                    )
                strip_tiles[(kb, bh)] = st

        # ---- pv + normalize + store ----
        for t in range(NB):
            ob = outs.tile([128, 128], fp32, tag="ob")
            for bh in range(2):
                kbs = [kb for kb in (t - 1, t, t + 1) if 0 <= kb < NB]
                pv = pv_psum.tile([128, 65], fp32, tag="pv")
                for j, kb in enumerate(kbs):
                    t0 = max(0, kb - 1)
                    off = (t - t0) * 128
                    nc.tensor.matmul(
                        pv,
                        strip_tiles[(kb, bh)][:, off : off + 128],
                        vp[bh][:, kb, :],
                        start=(j == 0),
                        stop=(j == len(kbs) - 1),
                    )
                r = small.tile([128, 1], fp32, tag="r")
                nc.vector.reciprocal(out=r, in_=pv[:, 64:65])
                nc.vector.tensor_scalar_mul(
                    ob[:, bh * 64 : bh * 64 + 64], pv[:, 0:64], r
                )
            nc.sync.dma_start(out=out[t * 128 : (t + 1) * 128, :], in_=ob)

    return nc


def kernel(query, key, value, window_size):
    assert int(window_size) == W
    from concourse.bass_utils import run_bass_kernel_spmd

    if "nc" not in _cached:
        _cached["nc"] = _build()
    nc = _cached["nc"]

    ident = np.eye(128, dtype=np.float32)
    kk, qq = np.arange(128)[:, None], np.arange(128)[None, :]
    m_band = np.concatenate(
        [(kk <= qq), np.ones((128, 128), bool), (kk >= qq)], axis=1
    ).astype(np.float16)

    q = np.asarray(query, np.float32)
    k = np.asarray(key, np.float32)
    v = np.asarray(value, np.float32)
    in_maps = []
    for c in range(8):
        b, h0 = c // 4, 2 * (c % 4)
        in_maps.append({
            "q": np.ascontiguousarray(q[b, :, h0 : h0 + 2, :]).reshape(S, 128),
            "k": np.ascontiguousarray(k[b, :, h0 : h0 + 2, :]).reshape(S, 128),
            "v": np.ascontiguousarray(v[b, :, h0 : h0 + 2, :]).reshape(S, 128),
            "ident": ident, "mband": m_band,
        })

    res = run_bass_kernel_spmd(nc, in_maps, list(range(8)))
    full = np.empty((B, S, H, D), np.float32)
    for c in range(8):
        b, h0 = c // 4, 2 * (c % 4)
        full[b, :, h0 : h0 + 2, :] = res.results[c]["out"].reshape(S, 2, D)
    return full


# revision 10
# speedup vs baseline: 1.9586x; 1.9586x over previous
"""Sliding-window attention (B=2,S=2048,H=8,D=64,W=128) on 8 trn2 cores.

Sharding: 16 (b,h) pairs -> 8 cores x 2 adjacent heads (same b). Per core the
inputs are [S, 2*64] fp32 slabs (adjacent heads => 512B contiguous DRAM runs).

Per-core kernel (all on-chip work in fp16, accumulation fp32):
  prep:   cast-DMA slabs to fp16; PE-transpose Q,K pairs -> QT/KT [128,2048]
          (rows 0:64 head0^T, 64:128 head1^T); build V' [128,16,65] per head
          with a ones column (row-sum trick for the softmax denominator).
  scores: per key-block kb: scoresT strip [128 k, <=384 q] = KT_blk.T @ QT_slab
          (one matmul, K=64 contraction); exp via ACT (scale=1/8 folded in),
          band masks as 0/1 triangular multiplies on the two edge sections.
  pv:     per query-block t: out[128,65] = sum_kb P^T_sec.T @ V'[kb] in PSUM;
          col 64 = softmax denom; normalize with DVE reciprocal + scalar mul.
"""

import numpy as np

B, S, H, D = 2, 2048, 8, 64
W = 128
NB = S // 128  # 16 seq blocks

_cached = {}


def _install_drain_split():
    """Walrus in this container encodes ~1 sync-wait per CTRL instruction; the
    Tile end-of-kernel drain aggregates one wait per live semaphore and fails
    codegen. Split the waits across single-wait NoOps on the sync engine."""
    import concourse.tile as tile
    from bass_rust import VectorClock, ScopedClock

    def _split_drain_and_barrier(self, tick_clock, wait_clock):
        gc = tick_clock.global_clock
        vals = [gc.peek_next(i) - 1 for i in range(27)]
        for i, v in [(i, v) for i, v in enumerate(vals) if v > 0]:
            sub = VectorClock()
            sub.require_at_least(i, v)
            nop_inst = self.nc.sync.nop(nofuse=True)
            wait_clock.add_sem_waits(nop_inst.ins, ScopedClock({None: sub}))
        self.nc.sync.drain()
        self.nc.all_engine_barrier()
        assert self.sems is not None
        popped = self.nc._tile_sem_poison_stack.pop()
        assert popped is self._sem_poison
        self.nc.clear_and_free_semaphores(list(self.sems.allocated().values()))
        self.nc.all_engine_barrier()

    tile.TileContext._drain_and_barrier = _split_drain_and_barrier


def _build():
    import concourse.bass as bass
    import concourse.mybir as mybir
    import concourse.tile as tile

    _install_drain_split()

    fp16 = mybir.dt.float16
    fp32 = mybir.dt.float32

    nc = bass.Bass()
    q_in = nc.dram_tensor("q", [S, 128], fp32, kind="ExternalInput")
    k_in = nc.dram_tensor("k", [S, 128], fp32, kind="ExternalInput")
    v_in = nc.dram_tensor("v", [S, 128], fp32, kind="ExternalInput")
    ident_in = nc.dram_tensor("ident", [128, 128], fp16, kind="ExternalInput")
    mle_in = nc.dram_tensor("mle", [128, 128], fp16, kind="ExternalInput")
    mge_in = nc.dram_tensor("mge", [128, 128], fp16, kind="ExternalInput")
    out = nc.dram_tensor("out", [S, 128], fp32, kind="ExternalOutput")

    with tile.TileContext(nc) as tc:
        consts = tc.tile_pool(name="consts", bufs=1).__enter__()
        stage = tc.tile_pool(name="stage", bufs=4).__enter__()
        tp_psum = tc.tile_pool(name="tp_psum", bufs=2, space="PSUM").__enter__()
        sc_psum = tc.tile_pool(name="sc_psum", bufs=3, space="PSUM").__enter__()
        pv_psum = tc.tile_pool(name="pv_psum", bufs=3, space="PSUM").__enter__()
        strips = tc.tile_pool(name="strips", bufs=12).__enter__()
        small = tc.tile_pool(name="small", bufs=4).__enter__()
        outs = tc.tile_pool(name="outs", bufs=3).__enter__()

        ident = consts.tile([128, 128], fp16, tag="ident")
        m_le = consts.tile([128, 128], fp16, tag="mle")
        m_ge = consts.tile([128, 128], fp16, tag="mge")
        nc.sync.dma_start(out=ident, in_=ident_in[:])
        nc.sync.dma_start(out=m_le, in_=mle_in[:])
        nc.sync.dma_start(out=m_ge, in_=mge_in[:])

        qt = consts.tile([128, S], fp16, tag="qt")
        kt = consts.tile([128, S], fp16, tag="kt")
        v_all = consts.tile([128, NB, 128], fp16, tag="v_all")
        vp = [consts.tile([128, NB, 65], fp16, tag=f"vp{bh}") for bh in range(2)]

        # ---- prep: cast loads + pair transposes ----
        for i in range(NB):
            sl = slice(i * 128, (i + 1) * 128)
            qs = stage.tile([128, 128], fp16, tag="qs")
            ks = stage.tile([128, 128], fp16, tag="ks")
            nc.gpsimd.dma_start(out=qs, in_=q_in[sl, :])
            nc.gpsimd.dma_start(out=ks, in_=k_in[sl, :])
            nc.gpsimd.dma_start(out=v_all[:, i, :], in_=v_in[sl, :])
            qtp = tp_psum.tile([128, 128], fp16, tag="tp")
            nc.tensor.transpose(qtp, qs, ident)
            nc.scalar.copy(out=qt[:, sl], in_=qtp)
            ktp = tp_psum.tile([128, 128], fp16, tag="tp")
            nc.tensor.transpose(ktp, ks, ident)
            nc.vector.tensor_copy(out=kt[:, sl], in_=ktp)
        for bh in range(2):
            nc.vector.tensor_copy(
                out=vp[bh][:, :, 0:64], in_=v_all[:, :, bh * 64 : bh * 64 + 64]
            )
            nc.vector.memset(vp[bh][:, :, 64:65], 1.0)

        # ---- scores: one strip per (kb, bh) ----
        # strip(kb) covers query blocks t in [t0, t1]; section offsets 128*(t-t0)
        strip_tiles = {}
        for kb in range(NB):
            t0, t1 = max(0, kb - 1), min(NB - 1, kb + 1)
            w = (t1 - t0 + 1) * 128
            c0, c1 = t0 // 4, t1 // 4
            for bh in range(2):
                rows = slice(bh * 64, bh * 64 + 64)
                sc = sc_psum.tile([128, 384], fp32, tag="sc")
                if c0 == c1:
                    nc.tensor.matmul(
                        sc[:, :w], ktb[kb][rows, :],
                        qtc[c0][rows, (t0 % 4) * 128 : (t0 % 4) * 128 + w],
                        start=True, stop=True,
                    )
                else:  # query slab crosses a qtc chunk boundary: two matmuls
                    w1 = (c1 * 4 - t0) * 128
                    nc.tensor.matmul(
                        sc[:, :w1], ktb[kb][rows, :],
                        qtc[c0][rows, (t0 % 4) * 128 : 512],
                        start=True, stop=True,
                    )
                    nc.tensor.matmul(
                        sc[:, w1:w], ktb[kb][rows, :],
                        qtc[c1][rows, 0 : w - w1],
                        start=True, stop=True,
                    )
                st = strips.tile([128, 384], fp16, tag="strip")
                nc.scalar.activation(
                    out=st[:, :w], in_=sc[:, :w],
                    func=mybir.ActivationFunctionType.Exp, scale=0.125,
                )
                if kb > 0:  # section t=kb-1: keep k <= q
                    nc.vector.tensor_mul(st[:, 0:128], st[:, 0:128], m_le)
                if kb < NB - 1:  # section t=kb+1: keep k >= q
                    off = (kb + 1 - t0) * 128
                    nc.vector.tensor_mul(
                        st[:, off : off + 128], st[:, off : off + 128], m_ge
                    )
                strip_tiles[(kb, bh)] = st

        # ---- pv + normalize + store ----
        for t in range(NB):
            ob = outs.tile([128, 128], fp32, tag="ob")
            for bh in range(2):
                kbs = [kb for kb in (t - 1, t, t + 1) if 0 <= kb < NB]
                pv = pv_psum.tile([128, 65], fp32, tag="pv")
                for j, kb in enumerate(kbs):
                    t0 = max(0, kb - 1)
                    off = (t - t0) * 128
                    nc.tensor.matmul(
                        pv,
                        strip_tiles[(kb, bh)][:, off : off + 128],
                        vp[bh][:, kb, :],
                        start=(j == 0),
                        stop=(j == len(kbs) - 1),
                    )
                r = small.tile([128, 1], fp32, tag="r")
                nc.vector.reciprocal(out=r, in_=pv[:, 64:65])
                nc.vector.tensor_scalar_mul(
                    ob[:, bh * 64 : bh * 64 + 64], pv[:, 0:64], r
                )
            nc.sync.dma_start(out=out[t * 128 : (t + 1) * 128, :], in_=ob)

    return nc


def kernel(query, key, value, window_size):
    assert int(window_size) == W
    from concourse.bass_utils import run_bass_kernel_spmd

    if "nc" not in _cached:
        _cached["nc"] = _build()
    nc = _cached["nc"]

    ident = np.eye(128, dtype=np.float16)
    kk, qq = np.arange(128)[:, None], np.arange(128)[None, :]
    m_le = (kk <= qq).astype(np.float16)
    m_ge = (kk >= qq).astype(np.float16)

    q = np.asarray(query, np.float32)
    k = np.asarray(key, np.float32)
    v = np.asarray(value, np.float32)
    in_maps = []
    for c in range(8):
        b, h0 = c // 4, 2 * (c % 4)
        in_maps.append({
            "q": np.ascontiguousarray(q[b, :, h0 : h0 + 2, :]).reshape(S, 128),
            "k": np.ascontiguousarray(k[b, :, h0 : h0 + 2, :]).reshape(S, 128),
            "v": np.ascontiguousarray(v[b, :, h0 : h0 + 2, :]).reshape(S, 128),
            "ident": ident, "mle": m_le, "mge": m_ge,
        })

    res = run_bass_kernel_spmd(nc, in_maps, list(range(8)))
    full = np.empty((B, S, H, D), np.float32)
    for c in range(8):
        b, h0 = c // 4, 2 * (c % 4)
        full[b, :, h0 : h0 + 2, :] = res.results[c]["out"].reshape(S, 2, D)
    return full
